# revision 1
# baseline (speedup 1.0000x reference)
"""Trainium2 Bass kernel for modality-routed (CogVLM-style) attention.

Contract: kernel(**inputs) takes FULL unsharded numpy inputs (as produced by
the reference's setup_inputs) and returns the FULL [2048, 4096] fp32 output.

Sharding: tensor-parallel over heads. Core r owns heads 4r..4r+3:
  - qkv weights column-sharded: q/k slices used as matmul lhsT (stationary),
    producing qT/kT directly in [D, S] orientation (no transposes on device);
    v computed in natural [S, d] orientation.
  - dense weights row-sharded [512, 4096]; each core emits a partial
    [2048, 4096] output, summed on the host (the unshard step).
Routing (vision tokens = rows 0..NV-1, language = NV..S-1) is handled by
splitting matmuls at the NV boundary; the vision qkv bias is fused into the
PSUM->SBUF evacuations.

All PE matmuls run in float32r (fp32 data, ~1.4e-4 rel err, full rate on TRN2
for N>=256). Softmax skips the max-subtraction (scores are O(10) here; exact
same math), computes probsT = exp(scoresT) tile-by-tile with causal block
skipping + a triangular mask on diagonal blocks, row sums via a ones-vector
matmul, and folds 1/sum into the attnT evacuation via a DMA-broadcast row.
"""

import sys

import numpy as np

if "/opt/trn_rl_repo" not in sys.path:
    sys.path.insert(0, "/opt/trn_rl_repo")

import concourse.bass as bass  # noqa: E402,F401
import concourse.tile as tile  # noqa: E402
from concourse import bacc, mybir  # noqa: E402
from concourse.bass_utils import run_bass_kernel_spmd  # noqa: E402

S = 2048
HID = 4096
H = 32
D = 128
NCORES = 8
HPC = H // NCORES          # heads per core = 4
QKC = 2 * HPC * D          # q+k outdim rows per core = 1024
VC = HPC * D               # v outdim per core = 512
NV = 576                   # vision tokens occupy rows [0, NV)
NKT = HID // 128           # 32 K-tiles

F32 = mybir.dt.float32
F32R = mybir.dt.float32r

_CACHE = {}


def _chunks():
    # (c0, c1, expert, side64): first vision chunk carries tokens 512..576
    # as a side segment so they share the same weight-tile stream; language
    # chunks are 128-aligned.
    return [(0, 512, "V", True), (NV, 1024, "L", False),
            (1024, 1536, "L", False), (1536, 2048, "L", False)]


def _token_tiles(t0, t1):
    out = []
    c = t0
    while c < t1:
        n = min(t1, (c // 128 + 1) * 128)
        out.append((c, n))
        c = n
    return out


def _build():
    nc = bacc.Bacc("TRN2", target_bir_lowering=False, debug=False,
                   num_devices=NCORES)
    dti = nc.dram_tensor
    hsT = dti("hsT", [HID, S], F32R, kind="ExternalInput").ap()
    wqk_v = dti("wqk_v", [HID, QKC], F32R, kind="ExternalInput").ap()
    wqk_l = dti("wqk_l", [HID, QKC], F32R, kind="ExternalInput").ap()
    wv_v = dti("wv_v", [HID, VC], F32R, kind="ExternalInput").ap()
    wv_l = dti("wv_l", [HID, VC], F32R, kind="ExternalInput").ap()
    wd_v = dti("wd_v", [VC, HID], F32R, kind="ExternalInput").ap()
    wd_l = dti("wd_l", [VC, HID], F32R, kind="ExternalInput").ap()
    bqk = dti("bqk", [128, 8], F32, kind="ExternalInput").ap()
    bv = dti("bv", [1, VC], F32, kind="ExternalInput").ap()
    cosq = dti("cosq", [D, S], F32, kind="ExternalInput").ap()
    sinq = dti("sinq", [D, S], F32, kind="ExternalInput").ap()
    cosk = dti("cosk", [D, S], F32, kind="ExternalInput").ap()
    sink = dti("sink", [D, S], F32, kind="ExternalInput").ap()
    rmT = dti("rmT", [D, D], F32R, kind="ExternalInput").ap()
    ones = dti("ones", [128, 1], F32R, kind="ExternalInput").ap()
    masks = dti("masks", [128, 4 * 512], F32R, kind="ExternalInput").ap()
    qkr_d = dti("qkr", [QKC, S], F32R).ap()          # roped qT/kT scratch
    v_d = dti("vsc", [S, VC], F32R).ap()             # v scratch
    rcp_d = dti("rcp", [HPC, S], F32).ap()           # softmax 1/sum rows
    out_d = dti("out", [S, HID], F32, kind="ExternalOutput").ap()

    CH = _chunks()
    with tile.TileContext(nc) as tc:
        with tc.tile_pool(name="glob", bufs=1) as glob:
            ones_t = glob.tile([128, 1], F32R)
            nc.sync.dma_start(out=ones_t[:], in_=ones[:])
            mask_t = glob.tile([128, 4 * 512], F32R)
            nc.sync.dma_start(out=mask_t[:], in_=masks[:])

            # ---------------- QKV phase ----------------
            with tc.tile_pool(name="consts", bufs=1) as consts, \
                 tc.tile_pool(name="hs", bufs=1) as hs_pool, \
                 tc.tile_pool(name="wq", bufs=2) as wq_pool, \
                 tc.tile_pool(name="wvp", bufs=1) as wv_pool, \
                 tc.tile_pool(name="ev", bufs=2) as ev_pool, \
                 tc.tile_pool(name="ps", bufs=2, space="PSUM") as ps, \
                 tc.tile_pool(name="ps1", bufs=2, space="PSUM") as ps1:
                bqk_t = consts.tile([128, 8], F32)
                nc.sync.dma_start(out=bqk_t[:], in_=bqk[:])
                bv_t = consts.tile([128, VC], F32)
                nc.sync.dma_start(out=bv_t[:], in_=bv[:].to_broadcast((128, VC)))
                rm_t = consts.tile([D, D], F32R)
                nc.sync.dma_start(out=rm_t[:], in_=rmT[:])

                for (c0, c1, e, side) in CH:
                    w = c1 - c0
                    cw = (NV - c0) if side else w       # cos/sin span
                    wqk = wqk_v if e == "V" else wqk_l
                    wv = wv_v if e == "V" else wv_l
                    hst = hs_pool.tile([128, NKT, 512], F32R, tag="hst")
                    for kt in range(NKT):
                        nc.sync.dma_start(
                            out=hst[:, kt, :w],
                            in_=hsT[128 * kt:128 * (kt + 1), c0:c1])
                    h64 = None
                    if side:
                        h64 = hs_pool.tile([128, NKT, 64], F32R, tag="h64")
                        for kt in range(NKT):
                            nc.sync.dma_start(
                                out=h64[:, kt, :],
                                in_=hsT[128 * kt:128 * (kt + 1), 512:NV])
                    cs = []
                    for tag, src in (("cqc", cosq), ("sqc", sinq),
                                     ("ckc", cosk), ("skc", sink)):
                        t = consts.tile([128, 576], F32, tag=tag)
                        nc.sync.dma_start(out=t[:, :cw], in_=src[:, c0:c0 + cw])
                        cs.append(t)
                    segs = [(c0, c1)] + ([(512, NV)] if side else [])
                    # q/k rows: m<HPC -> q head m, m>=HPC -> k head m-HPC
                    for m in range(2 * HPC):
                        wt = wq_pool.tile([128, NKT, 128], F32R, tag="wt")
                        for kt in range(NKT):
                            nc.sync.dma_start(
                                out=wt[:, kt, :],
                                in_=wqk[128 * kt:128 * (kt + 1),
                                        128 * m:128 * (m + 1)])
                        cos_c = cs[0] if m < HPC else cs[2]
                        sin_c = cs[1] if m < HPC else cs[3]
                        for (a0, a1) in segs:
                            w2 = a1 - a0
                            is64 = side and a0 >= 512
                            src = h64 if is64 else hst
                            o0 = a0 - c0
                            pt = ps.tile([128, 512], F32, tag="qk_ps")
                            for kt in range(NKT):
                                nc.tensor.matmul(pt[:, :w2], wt[:, kt, :],
                                                 src[:, kt, :w2],
                                                 start=(kt == 0),
                                                 stop=(kt == NKT - 1))
                            qk_sb = ev_pool.tile([128, 512], F32R, tag="qk_sb")
                            if e == "V":
                                nc.scalar.activation(
                                    out=qk_sb[:, :w2], in_=pt[:, :w2],
                                    func=mybir.ActivationFunctionType.Identity,
                                    bias=bqk_t[:, m:m + 1], scale=1.0)
                            else:
                                nc.scalar.activation(
                                    out=qk_sb[:, :w2], in_=pt[:, :w2],
                                    func=mybir.ActivationFunctionType.Copy,
                                    scale=1.0)
                            rot = ps1.tile([128, 512], F32, tag="rot_ps")
                            nc.tensor.matmul(rot[:, :w2], rm_t[:],
                                             qk_sb[:, :w2],
                                             start=True, stop=True)
                            tb = ev_pool.tile([128, 512], F32, tag="tb")
                            nc.vector.tensor_mul(tb[:, :w2], rot[:, :w2],
                                                 sin_c[:, o0:o0 + w2])
                            nc.vector.tensor_mul(qk_sb[:, :w2], qk_sb[:, :w2],
                                                 cos_c[:, o0:o0 + w2])
                            rr = ev_pool.tile([128, 512], F32R, tag="rr")
                            nc.vector.tensor_add(rr[:, :w2], qk_sb[:, :w2],
                                                 tb[:, :w2])
                            nc.sync.dma_start(
                                out=qkr_d[128 * m:128 * (m + 1), a0:a1],
                                in_=rr[:, :w2])
                    # v in natural [token, d] orientation
                    wvt = wv_pool.tile([128, NKT, VC], F32R, tag="wvt")
                    for kt in range(NKT):
                        nc.sync.dma_start(
                            out=wvt[:, kt, :],
                            in_=wv[128 * kt:128 * (kt + 1), :])
                    tts = _token_tiles(c0, c1) + ([(512, NV)] if side else [])
                    for (t0, t1) in tts:
                        mw = t1 - t0
                        src = h64 if t0 >= 512 and side else hst
                        off = 512 if (t0 >= 512 and side) else c0
                        pv = ps.tile([128, VC], F32, tag="v_ps")
                        for kt in range(NKT):
                            nc.tensor.matmul(
                                pv[:mw, :],
                                src[:, kt, t0 - off:t1 - off],
                                wvt[:, kt, :],
                                start=(kt == 0), stop=(kt == NKT - 1))
                        v_sb = ev_pool.tile([128, VC], F32R, tag="v_sb")
                        if e == "V":
                            nc.vector.tensor_add(v_sb[:mw, :], pv[:mw, :],
                                                 bv_t[:mw, :])
                        else:
                            nc.vector.tensor_copy(v_sb[:mw, :], pv[:mw, :])
                        nc.sync.dma_start(out=v_d[t0:t1, :], in_=v_sb[:mw, :])

            # ---------------- attention phase ----------------
            with tc.tile_pool(name="an", bufs=1) as an_pool:
                attn = []
                for h in range(HPC):
                    a_t = an_pool.tile([128, S], F32R, tag=f"attnT{h}")
                    attn.append(a_t)
                with tc.tile_pool(name="att", bufs=1) as att, \
                     tc.tile_pool(name="pr", bufs=3) as pr_pool, \
                     tc.tile_pool(name="sm", bufs=2) as sm_pool, \
                     tc.tile_pool(name="ps2", bufs=3, space="PSUM") as ps2, \
                     tc.tile_pool(name="ps3", bufs=2, space="PSUM") as ps3, \
                     tc.tile_pool(name="ps4", bufs=2, space="PSUM") as ps4:
                    for h in range(HPC):
                        qt = att.tile([128, S], F32R, tag="qt")
                        kt_ = att.tile([128, S], F32R, tag="kt")
                        vt = att.tile([128, 16, 128], F32R, tag="vt")
                        nc.sync.dma_start(
                            out=qt[:], in_=qkr_d[128 * h:128 * (h + 1), :])
                        nc.sync.dma_start(
                            out=kt_[:],
                            in_=qkr_d[VC + 128 * h:VC + 128 * (h + 1), :])
                        for tt in range(16):
                            nc.sync.dma_start(
                                out=vt[:, tt, :],
                                in_=v_d[128 * tt:128 * (tt + 1),
                                        128 * h:128 * (h + 1)])
                        for c in range(4):
                            nj = 4 * c + 4
                            ap = ps3.tile([128, 512], F32, tag="attn_ps")
                            sp = ps4.tile([1, 512], F32, tag="sum_ps")
                            for j in range(nj):
                                scp = ps2.tile([128, 512], F32, tag="sc_ps")
                                nc.tensor.matmul(
                                    scp[:], kt_[:, 128 * j:128 * (j + 1)],
                                    qt[:, 512 * c:512 * (c + 1)],
                                    start=True, stop=True)
                                pb = pr_pool.tile([128, 512], F32R, tag="probs")
                                nc.scalar.activation(
                                    out=pb[:], in_=scp[:],
                                    func=mybir.ActivationFunctionType.Exp,
                                    scale=1.0)
                                if j >= 4 * c:
                                    r = j - 4 * c
                                    nc.vector.tensor_mul(
                                        pb[:], pb[:],
                                        mask_t[:, 512 * r:512 * (r + 1)])
                                nc.tensor.matmul(sp[:], ones_t[:], pb[:],
                                                 start=(j == 0),
                                                 stop=(j == nj - 1))
                                nc.tensor.matmul(ap[:], vt[:, j, :], pb[:],
                                                 start=(j == 0),
                                                 stop=(j == nj - 1))
                            rc = sm_pool.tile([1, 512], F32, tag="rc")
                            nc.vector.reciprocal(rc[:], sp[:])
                            nc.sync.dma_start(
                                out=rcp_d[h:h + 1, 512 * c:512 * (c + 1)],
                                in_=rc[:])
                            rb = sm_pool.tile([128, 512], F32, tag="rb")
                            nc.sync.dma_start(
                                out=rb[:],
                                in_=rcp_d[h:h + 1, 512 * c:512 * (c + 1)]
                                .to_broadcast((128, 512)))
                            nc.vector.tensor_mul(
                                attn[h][:, 512 * c:512 * (c + 1)], ap[:],
                                rb[:])

                # ---------------- dense phase ----------------
                with tc.tile_pool(name="wd", bufs=2) as wd_pool, \
                     tc.tile_pool(name="oe", bufs=3) as oe_pool, \
                     tc.tile_pool(name="ps5", bufs=3, space="PSUM") as ps5:
                    ranges = []
                    for (t0, t1) in _token_tiles(0, S):
                        if t0 < NV < t1:
                            ranges.append((t0, NV, "V"))
                            ranges.append((NV, t1, "L"))
                        else:
                            ranges.append((t0, t1, "V" if t0 < NV else "L"))
                    for nh in range(2):
                        wdv_t = wd_pool.tile([128, HPC, 2048], F32R, tag="wdv")
                        wdl_t = wd_pool.tile([128, HPC, 2048], F32R, tag="wdl")
                        for hh in range(HPC):
                            nc.sync.dma_start(
                                out=wdv_t[:, hh, :],
                                in_=wd_v[128 * hh:128 * (hh + 1),
                                         2048 * nh:2048 * (nh + 1)])
                            nc.sync.dma_start(
                                out=wdl_t[:, hh, :],
                                in_=wd_l[128 * hh:128 * (hh + 1),
                                         2048 * nh:2048 * (nh + 1)])
                        for (t0, t1, e) in ranges:
                            mw = t1 - t0
                            wd_t = wdv_t if e == "V" else wdl_t
                            for n in range(4):
                                po = ps5.tile([128, 512], F32, tag="o_ps")
                                for hh in range(HPC):
                                    nc.tensor.matmul(
                                        po[:mw, :],
                                        attn[hh][:, t0:t1],
                                        wd_t[:, hh, 512 * n:512 * (n + 1)],
                                        start=(hh == 0), stop=(hh == HPC - 1))
                                ob = oe_pool.tile([128, 512], F32, tag="ob")
                                nc.vector.tensor_copy(ob[:mw, :], po[:mw, :])
                                nc.sync.dma_start(
                                    out=out_d[t0:t1,
                                              2048 * nh + 512 * n:
                                              2048 * nh + 512 * (n + 1)],
                                    in_=ob[:mw, :])
    nc.compile()
    return nc


def _prep_inputs(inputs):
    hs = np.ascontiguousarray(inputs["hidden_states"], np.float32)
    cos = np.asarray(inputs["cos"], np.float32)
    sin = np.asarray(inputs["sin"], np.float32)
    vi = np.asarray(inputs["vision_indices"]).ravel()
    li = np.asarray(inputs["language_indices"]).ravel()
    nv = vi.size
    assert nv == NV and np.array_equal(vi, np.arange(nv)) and \
        np.array_equal(li, np.arange(nv, S)), "unsupported index layout"
    hsT = np.ascontiguousarray(hs.T)
    scale = 1.0 / np.sqrt(np.float32(D))
    cosT = np.ascontiguousarray(cos.T)
    sinT = np.ascontiguousarray(sin.T)
    rmT = np.zeros((D, D), np.float32)
    for d in range(64):
        rmT[d + 64, d] = -1.0
        rmT[d, d + 64] = 1.0
    masks = np.zeros((128, 4 * 512), np.float32)
    tri = np.tril(np.ones((128, 128), np.float32)).T  # [t, s]: 1 iff t <= s
    for r in range(4):
        blk = np.ones((128, 512), np.float32)
        blk[:, :128 * r] = 0.0
        blk[:, 128 * r:128 * (r + 1)] = tri
        masks[:, 512 * r:512 * (r + 1)] = blk
    b = np.asarray(inputs["vision_qkv_b"], np.float32)
    in_maps = []
    for r in range(NCORES):
        h0 = HPC * r
        qc = slice(128 * h0, 128 * h0 + VC)
        kc = slice(HID + 128 * h0, HID + 128 * h0 + VC)
        vcs = slice(2 * HID + 128 * h0, 2 * HID + 128 * h0 + VC)
        wqk_vv = np.concatenate([inputs["vision_qkv_w"][:, qc],
                                 inputs["vision_qkv_w"][:, kc]], 1)
        wqk_ll = np.concatenate([inputs["lang_qkv_w"][:, qc],
                                 inputs["lang_qkv_w"][:, kc]], 1)
        bqk_r = np.concatenate([b[qc], b[kc]]).reshape(8, 128).T
        in_maps.append({
            "hsT": hsT,
            "wqk_v": np.ascontiguousarray(wqk_vv, np.float32),
            "wqk_l": np.ascontiguousarray(wqk_ll, np.float32),
            "wv_v": np.ascontiguousarray(inputs["vision_qkv_w"][:, vcs],
                                         np.float32),
            "wv_l": np.ascontiguousarray(inputs["lang_qkv_w"][:, vcs],
                                         np.float32),
            "wd_v": np.ascontiguousarray(
                inputs["vision_dense_w"][128 * h0:128 * h0 + VC, :],
                np.float32),
            "wd_l": np.ascontiguousarray(
                inputs["lang_dense_w"][128 * h0:128 * h0 + VC, :], np.float32),
            "bqk": np.ascontiguousarray(bqk_r, np.float32),
            "bv": np.ascontiguousarray(b[vcs].reshape(1, VC), np.float32),
            "cosq": cosT * scale, "sinq": sinT * scale,
            "cosk": cosT, "sink": sinT,
            "rmT": rmT, "ones": np.ones((128, 1), np.float32),
            "masks": masks,
        })
    return in_maps


def kernel(**inputs):
    if "nc" not in _CACHE:
        _CACHE["nc"] = _build()
    nc = _CACHE["nc"]
    in_maps = _prep_inputs(inputs)
    res = run_bass_kernel_spmd(nc, in_maps, list(range(NCORES)),
                               **_CACHE.get("run_kwargs", {}))
    _CACHE["last_results"] = res
    out = np.zeros((S, HID), np.float64)
    for r in range(NCORES):
        out += res.results[r]["out"].astype(np.float64)
    return out.astype(np.float32)



# revision 9
# speedup vs baseline: 1.7782x; 1.7782x over previous
"""Trainium2 Bass kernel for modality-routed (CogVLM-style) attention.

Contract: kernel(**inputs) takes FULL unsharded numpy inputs (as produced by
the reference's setup_inputs) and returns the FULL [2048, 4096] fp32 output.

Sharding: tensor-parallel over heads. Core r owns heads 4r..4r+3:
  - qkv weights column-sharded; q/k computed weight-stationary producing
    qT/kT [d, tok] directly, v computed token-stationary in [tok, d].
  - dense weights row-sharded [512, 4096]; each core emits a partial
    [2048, 4096] fp32 output, summed on the host (the unshard step).

v2: everything in bf16 on the PE (fp32 only in PSUM accumulation and the
final output partials). All intermediates (qT/kT, v, attn) stay in SBUF —
no DRAM scratch roundtrips. Weights are pre-tiled on the host into the
exact SBUF layouts so every weight load is one contiguous-per-partition
DMA. Softmax reciprocal runs after a [1,512]->[128,512] broadcast (128x
faster on DVE than on one partition). Routing (vision tokens = rows
0..NV-1) is handled by splitting matmuls at the NV boundary.
"""

import math
import sys

import numpy as np

if "/opt/trn_rl_repo" not in sys.path:
    sys.path.insert(0, "/opt/trn_rl_repo")

import ml_dtypes  # noqa: E402

import concourse.bass as bass  # noqa: E402,F401
import concourse.tile as tile  # noqa: E402
from concourse import bacc, mybir  # noqa: E402
from concourse.bass_utils import run_bass_kernel_spmd  # noqa: E402

S = 2048
HID = 4096
H = 32
D = 128
NCORES = 8
HPC = H // NCORES          # heads per core = 4
VC = HPC * D               # per-core q (or k or v) width = 512
NV = 576                   # vision tokens occupy rows [0, NV)
NKT = HID // 128           # 32 contraction tiles
SCALE = 1.0 / math.sqrt(D)

BF = mybir.dt.bfloat16
F32 = mybir.dt.float32
BF_NP = ml_dtypes.bfloat16

_CACHE = {}

# (c0, c1, expert, side64): first vision chunk carries tokens 512..576 as a
# side segment so they share the same weight-tile stream; language chunks
# are bounded by PSUM width (512).
CHUNKS = [(0, 512, "V", True), (NV, 1024, "L", False),
          (1024, 1536, "L", False), (1536, 2048, "L", False)]


def _token_tiles(t0, t1):
    out = []
    c = t0
    while c < t1:
        n = min(t1, (c // 128 + 1) * 128)
        out.append((c, n))
        c = n
    return out


def _build():
    nc = bacc.Bacc("TRN2", target_bir_lowering=False, debug=False,
                   num_devices=NCORES)
    dti = nc.dram_tensor
    hsw = dti("hsw", [128, NKT, S], BF, kind="ExternalInput").ap()
    wqk_v = dti("wqk_v", [8, 128, NKT, 128], BF, kind="ExternalInput").ap()
    wqk_l = dti("wqk_l", [8, 128, NKT, 128], BF, kind="ExternalInput").ap()
    wv_v = dti("wv_v", [128, NKT, VC], BF, kind="ExternalInput").ap()
    wv_l = dti("wv_l", [128, NKT, VC], BF, kind="ExternalInput").ap()
    wdw = dti("wdw", [2, 128, 2, HPC, 2048], BF, kind="ExternalInput").ap()
    bqk = dti("bqk", [128, 8], F32, kind="ExternalInput").ap()
    bv = dti("bv", [1, VC], F32, kind="ExternalInput").ap()
    cosw = dti("cosw", [D, S], BF, kind="ExternalInput").ap()
    sinw = dti("sinw", [D, S], BF, kind="ExternalInput").ap()
    rmT = dti("rmT", [D, D], BF, kind="ExternalInput").ap()
    ones = dti("ones", [128, 128], BF, kind="ExternalInput").ap()
    masks = dti("masks", [128, 4, 512], BF, kind="ExternalInput").ap()
    out_d = dti("out", [S, HID], F32, kind="ExternalOutput").ap()

    with tile.TileContext(nc) as tc:
        with tc.tile_pool(name="glob", bufs=1) as glob:
            qk = glob.tile([128, 8, S], BF)        # qT (m 0..3) / kT (m 4..7)
            vsb = glob.tile([128, 16, VC], BF)     # v[128t+p, :] token tiles
            cos_t = glob.tile([D, S], BF)
            nc.sync.dma_start(out=cos_t[:], in_=cosw[:])
            sin_t = glob.tile([D, S], BF)
            nc.sync.dma_start(out=sin_t[:], in_=sinw[:])
            mask_t = glob.tile([128, 4, 512], BF)
            nc.sync.dma_start(out=mask_t[:], in_=masks[:])
            ones_t = glob.tile([128, 128], BF)
            nc.sync.dma_start(out=ones_t[:], in_=ones[:])
            rm_t = glob.tile([D, D], BF)
            nc.sync.dma_start(out=rm_t[:], in_=rmT[:])
            bqk_t = glob.tile([128, 8], F32)
            nc.sync.dma_start(out=bqk_t[:], in_=bqk[:])
            bv_t = glob.tile([128, VC], F32)
            nc.sync.dma_start(out=bv_t[:], in_=bv[:].to_broadcast((128, VC)))

            # ---------------- QKV phase ----------------
            with tc.tile_pool(name="hs", bufs=2) as hs_pool, \
                 tc.tile_pool(name="h64", bufs=1) as h64_pool, \
                 tc.tile_pool(name="wq", bufs=2) as wq_pool, \
                 tc.tile_pool(name="wvp", bufs=1) as wv_pool, \
                 tc.tile_pool(name="ev", bufs=3) as ev_pool, \
                 tc.tile_pool(name="psA", bufs=2, space="PSUM") as psA, \
                 tc.tile_pool(name="ps64", bufs=2, space="PSUM") as ps64, \
                 tc.tile_pool(name="psR", bufs=2, space="PSUM") as psR:
                for (c0, c1, e, side) in CHUNKS:
                    w = c1 - c0
                    wqk = wqk_v if e == "V" else wqk_l
                    wv = wv_v if e == "V" else wv_l
                    hst = hs_pool.tile([128, NKT, 512], BF, tag="hst")
                    nc.sync.dma_start(out=hst[:, :, :w], in_=hsw[:, :, c0:c1])
                    h64 = None
                    if side:
                        h64 = h64_pool.tile([128, NKT, 64], BF, tag="h64")
                        nc.sync.dma_start(out=h64[:], in_=hsw[:, :, 512:NV])
                    # --- q/k, weight-stationary -> qT/kT [d, tok] + RoPE ---
                    for m in range(8):
                        wt = wq_pool.tile([128, NKT, 128], BF, tag="wt")
                        nc.sync.dma_start(out=wt[:], in_=wqk[m])
                        pt = psA.tile([128, 512], F32, tag="pt")
                        p64 = None
                        if side:
                            p64 = ps64.tile([128, 64], F32, tag="p64")
                        for kt in range(NKT):
                            nc.tensor.matmul(pt[:, :w], wt[:, kt, :],
                                             hst[:, kt, :w],
                                             start=(kt == 0),
                                             stop=(kt == NKT - 1))
                            if side:
                                nc.tensor.matmul(p64[:], wt[:, kt, :],
                                                 h64[:, kt, :],
                                                 start=(kt == 0),
                                                 stop=(kt == NKT - 1))
                        segs = [(c0, c1, pt)] + ([(512, NV, p64)] if side
                                                 else [])
                        for (a0, a1, src_ps) in segs:
                            ww = a1 - a0
                            qs = ev_pool.tile([128, 512], BF, tag="qs")
                            if e == "V":
                                nc.scalar.activation(
                                    out=qs[:, :ww], in_=src_ps[:, :ww],
                                    func=mybir.ActivationFunctionType.Identity,
                                    bias=bqk_t[:, m:m + 1], scale=1.0)
                            else:
                                nc.scalar.activation(
                                    out=qs[:, :ww], in_=src_ps[:, :ww],
                                    func=mybir.ActivationFunctionType.Copy,
                                    scale=1.0)
                            rot = psR.tile([128, 512], F32, tag="rot")
                            nc.tensor.matmul(rot[:, :ww], rm_t[:],
                                             qs[:, :ww], start=True, stop=True)
                            tb = ev_pool.tile([128, 512], BF, tag="tb")
                            nc.vector.tensor_mul(tb[:, :ww], rot[:, :ww],
                                                 sin_t[:, a0:a1])
                            qc = ev_pool.tile([128, 512], BF, tag="qc")
                            nc.vector.tensor_mul(qc[:, :ww], qs[:, :ww],
                                                 cos_t[:, a0:a1])
                            nc.vector.tensor_add(qk[:, m, a0:a1], qc[:, :ww],
                                                 tb[:, :ww])
                    # --- v, token-stationary -> v [tok, d] ---
                    wvt = wv_pool.tile([128, NKT, VC], BF, tag="wvt")
                    nc.sync.dma_start(out=wvt[:], in_=wv[:])
                    tts = _token_tiles(c0, c1) + ([(512, NV)] if side else [])
                    for (t0, t1) in tts:
                        mw = t1 - t0
                        is64 = side and t0 >= 512
                        src = h64 if is64 else hst
                        off = 512 if is64 else c0
                        pv = psA.tile([128, 512], F32, tag="pt")
                        for kt in range(NKT):
                            nc.tensor.matmul(
                                pv[:mw, :], src[:, kt, t0 - off:t1 - off],
                                wvt[:, kt, :],
                                start=(kt == 0), stop=(kt == NKT - 1))
                        tt, po = t0 // 128, t0 % 128
                        if po == 0:
                            if e == "V":
                                nc.vector.tensor_add(vsb[:mw, tt, :],
                                                     pv[:mw, :], bv_t[:mw, :])
                            else:
                                nc.scalar.activation(
                                    out=vsb[:mw, tt, :], in_=pv[:mw, :],
                                    func=mybir.ActivationFunctionType.Copy,
                                    scale=1.0)
                        else:
                            # tokens 576..640: partition-offset fixup via DMA
                            vs = ev_pool.tile([128, 512], BF, tag="vs")
                            nc.scalar.activation(
                                out=vs[:mw, :], in_=pv[:mw, :],
                                func=mybir.ActivationFunctionType.Copy,
                                scale=1.0)
                            nc.gpsimd.dma_start(out=vsb[po:po + mw, tt, :],
                                                in_=vs[:mw, :])

            # ---------------- attention phase ----------------
            with tc.tile_pool(name="an", bufs=1) as an_pool:
                attn = an_pool.tile([128, HPC, S], BF)
                with tc.tile_pool(name="pr", bufs=3) as pr_pool, \
                     tc.tile_pool(name="sm", bufs=2) as sm_pool, \
                     tc.tile_pool(name="psS", bufs=3, space="PSUM") as psS, \
                     tc.tile_pool(name="psP", bufs=2, space="PSUM") as psP, \
                     tc.tile_pool(name="psU", bufs=2, space="PSUM") as psU:
                    for h in range(HPC):
                        for c in range(4):
                            nj = 4 * c + 4
                            ap_ps = psP.tile([128, 512], F32, tag="ap")
                            sp_ps = psU.tile([128, 512], F32, tag="sp")
                            for j in range(nj):
                                scp = psS.tile([128, 512], F32, tag="sc")
                                nc.tensor.matmul(
                                    scp[:],
                                    qk[:, 4 + h, 128 * j:128 * (j + 1)],
                                    qk[:, h, 512 * c:512 * (c + 1)],
                                    start=True, stop=True)
                                pb = pr_pool.tile([128, 512], BF, tag="pb")
                                nc.scalar.activation(
                                    out=pb[:], in_=scp[:],
                                    func=mybir.ActivationFunctionType.Exp,
                                    scale=SCALE)
                                if j >= 4 * c:
                                    nc.vector.tensor_mul(
                                        pb[:], pb[:],
                                        mask_t[:, j - 4 * c, :])
                                nc.tensor.matmul(sp_ps[:], ones_t[:], pb[:],
                                                 start=(j == 0),
                                                 stop=(j == nj - 1))
                                nc.tensor.matmul(
                                    ap_ps[:],
                                    vsb[:, j, 128 * h:128 * (h + 1)], pb[:],
                                    start=(j == 0), stop=(j == nj - 1))
                            rb = sm_pool.tile([128, 512], F32, tag="rb")
                            nc.vector.reciprocal(rb[:], sp_ps[:])
                            nc.vector.tensor_mul(
                                attn[:, h, 512 * c:512 * (c + 1)],
                                ap_ps[:], rb[:])

                # ---------------- dense phase ----------------
                with tc.tile_pool(name="wd", bufs=1) as wd_pool, \
                     tc.tile_pool(name="oe", bufs=2) as oe_pool, \
                     tc.tile_pool(name="psD", bufs=2, space="PSUM") as psD:
                    wd_t = wd_pool.tile([128, 2, 2, HPC, 2048], BF, tag="wd")
                    nc.sync.dma_start(out=wd_t[:, 0], in_=wdw[0])
                    nc.sync.dma_start(out=wd_t[:, 1], in_=wdw[1])
                    ranges = []
                    for (t0, t1) in _token_tiles(0, S):
                        if t0 < NV < t1:
                            ranges.append((t0, NV, 0))
                            ranges.append((NV, t1, 1))
                        else:
                            ranges.append((t0, t1, 0 if t0 < NV else 1))
                    for (t0, t1, ei) in ranges:
                        mw = t1 - t0
                        ob = oe_pool.tile([128, HID], F32, tag="ob")
                        for nh in range(2):
                            pos = []
                            for n in range(4):
                                po_n = psD.tile([128, 512], F32, tag=f"po{n}")
                                pos.append(po_n)
                            for hh in range(HPC):
                                for n in range(4):
                                    nc.tensor.matmul(
                                        pos[n][:mw, :],
                                        attn[:, hh, t0:t1],
                                        wd_t[:, ei, nh, hh,
                                             512 * n:512 * (n + 1)],
                                        start=(hh == 0), stop=(hh == HPC - 1))
                            for n in range(4):
                                dst = ob[:mw, 2048 * nh + 512 * n:
                                         2048 * nh + 512 * (n + 1)]
                                if n % 2 == 0:
                                    nc.scalar.activation(
                                        out=dst, in_=pos[n][:mw, :],
                                        func=mybir.ActivationFunctionType.Copy,
                                        scale=1.0)
                                else:
                                    nc.vector.tensor_copy(dst, pos[n][:mw, :])
                        nc.sync.dma_start(out=out_d[t0:t1, :], in_=ob[:mw, :])
    nc.compile()
    return nc


def _prep_inputs(inputs):
    hs = np.asarray(inputs["hidden_states"], np.float32)
    cos = np.asarray(inputs["cos"], np.float32)
    sin = np.asarray(inputs["sin"], np.float32)
    vi = np.asarray(inputs["vision_indices"]).ravel()
    li = np.asarray(inputs["language_indices"]).ravel()
    nv = vi.size
    assert nv == NV and np.array_equal(vi, np.arange(nv)) and \
        np.array_equal(li, np.arange(nv, S)), "unsupported index layout"

    # hsT tiled: hsw[p, kt, t] = hs[t, 128*kt + p]
    hsw = np.ascontiguousarray(
        hs.T.reshape(NKT, 128, S).transpose(1, 0, 2)).astype(BF_NP)
    cosT = np.ascontiguousarray(cos.T).astype(BF_NP)
    sinT = np.ascontiguousarray(sin.T).astype(BF_NP)
    rmT = np.zeros((D, D), np.float32)
    for d in range(64):
        rmT[d + 64, d] = -1.0
        rmT[d, d + 64] = 1.0
    masks = np.zeros((128, 4, 512), np.float32)
    tri = np.tril(np.ones((128, 128), np.float32)).T  # [t, s]: 1 iff t <= s
    for r in range(4):
        blk = np.ones((128, 512), np.float32)
        blk[:, :128 * r] = 0.0
        blk[:, 128 * r:128 * (r + 1)] = tri
        masks[:, r, :] = blk
    b = np.asarray(inputs["vision_qkv_b"], np.float32)
    wq_all = {"V": np.asarray(inputs["vision_qkv_w"], np.float32),
              "L": np.asarray(inputs["lang_qkv_w"], np.float32)}
    wd_all = {"V": np.asarray(inputs["vision_dense_w"], np.float32),
              "L": np.asarray(inputs["lang_dense_w"], np.float32)}

    def qk_tiles(W, r):
        # [8, 128, NKT, 128]: m 0..3 q-heads, 4..7 k-heads of core r
        cols = []
        for m in range(8):
            col0 = (0 if m < 4 else HID) + VC * r + 128 * (m % 4)
            cols.append(W[:, col0:col0 + 128])
        arr = np.stack(cols, 0)                    # [8, HID, 128]
        return np.ascontiguousarray(
            arr.reshape(8, NKT, 128, 128).transpose(0, 2, 1, 3)).astype(BF_NP)

    def v_tiles(W, r):
        # [128, NKT, VC]
        c0 = 2 * HID + VC * r
        return np.ascontiguousarray(
            W[:, c0:c0 + VC].reshape(NKT, 128, VC).transpose(1, 0, 2)
        ).astype(BF_NP)

    def d_tiles(Wv, Wl, r):
        # [2(expert), 128, 2(nh), HPC, 2048]
        out = np.empty((2, 128, 2, HPC, 2048), np.float32)
        for ei, W in enumerate((Wv, Wl)):
            rows = W[VC * r:VC * r + VC, :]        # [512, 4096]
            blk = rows.reshape(HPC, 128, 2, 2048)  # [hh, p, nh, c]
            out[ei] = blk.transpose(1, 2, 0, 3)
        return np.ascontiguousarray(out).astype(BF_NP)

    in_maps = []
    for r in range(NCORES):
        bqk_r = np.empty((128, 8), np.float32)
        for m in range(8):
            col0 = (0 if m < 4 else HID) + VC * r + 128 * (m % 4)
            bqk_r[:, m] = b[col0:col0 + 128]
        in_maps.append({
            "hsw": hsw,
            "wqk_v": qk_tiles(wq_all["V"], r),
            "wqk_l": qk_tiles(wq_all["L"], r),
            "wv_v": v_tiles(wq_all["V"], r),
            "wv_l": v_tiles(wq_all["L"], r),
            "wdw": d_tiles(wd_all["V"], wd_all["L"], r),
            "bqk": bqk_r,
            "bv": np.ascontiguousarray(
                b[2 * HID + VC * r:2 * HID + VC * r + VC].reshape(1, VC)),
            "cosw": cosT, "sinw": sinT,
            "rmT": rmT.astype(BF_NP),
            "ones": np.ones((128, 128), BF_NP),
            "masks": masks.astype(BF_NP),
        })
    return in_maps


def kernel(**inputs):
    if "nc" not in _CACHE:
        _CACHE["nc"] = _build()
    nc = _CACHE["nc"]
    in_maps = _prep_inputs(inputs)
    res = run_bass_kernel_spmd(nc, in_maps, list(range(NCORES)),
                               **_CACHE.get("run_kwargs", {}))
    _CACHE["last_results"] = res
    out = np.zeros((S, HID), np.float64)
    for r in range(NCORES):
        out += res.results[r]["out"].astype(np.float64)
    return out.astype(np.float32)
